# revision 19
# baseline (speedup 1.0000x reference)
"""Fused sparse-attention kernel for TRN2, SPMD over 8 NeuronCores.

Sharding: data-parallel over batch (32 -> 4 per core). Per core, the full
block (LayerNorm -> fused qkv -> per-head attention with gathered relative
position bias -> proj) is computed on-chip; attention probabilities never
touch HBM.

v2 engine-balance restructure (baseline was PE-bound at 89% busy with ACT
at 66%):

* Scores are emitted four heads at a time into four distinct 32-row PE
  strips (tile_position row groups), so the K=16 QK^T matmuls stream
  concurrently instead of serially (the baseline interleaved full-row
  identity matmuls between them, which serialized everything).
* The gathered relative-position bias is split by head group: heads 0-3
  keep the additive-B identity-matmul accumulation into the score PSUM
  (PE), heads 4-7 multiply exp(B) into the exp'd probabilities on the DVE
  (f16 tensor_tensor at 2x rate). This splits ~110us of bias work across
  two engines instead of loading either fully.
* ACT runs only exp (fused [128,1024] two-head tiles) plus the tiny LN
  sqrt; PSUM drains move to GpSimd, the qkv-bias drain to the DVE.
* Softmax normalization: ones-column in V gives row sums in PSUM row 64;
  a per-head reciprocal_approx_fast writes a [1,512] f16 row at partition
  base 0, GpSimd broadcasts it, DVE multiplies (f16 2x).
* The output projection for batch b is emitted right after its attention
  completes (inside the last head-group pass) and borrows the freed PV
  PSUM banks, so proj matmuls fill PE gaps while ACT streams the next
  batch's exps.
"""

import os
import sys

import numpy as np

for _p in ("/opt/trn_rl_repo", "/root/.axon_site/_ro/trn_rl_repo"):
    if os.path.isdir(_p) and _p not in sys.path:
        sys.path.insert(0, _p)

import concourse.bacc as bacc
import concourse.tile as tile
from concourse import bass_utils, mybir
from concourse.masks import make_identity

F32 = mybir.dt.float32
F16 = mybir.dt.float16

NCORES = 8
B_TOTAL = 32
NB = B_TOTAL // NCORES  # local batch per core
N = 1024
NT = 8        # 128-row tiles over n
DIM = 256
CC = 2        # 128-row chunks over DIM
H = 8
KD = 16
D = 64
MC = 8        # 128-row chunks over m
EPS = 1e-5
OFF = float(4.0 * np.log(2.0))  # exp offset for fp16 headroom (cancels)

MULT = mybir.AluOpType.mult
ADD = mybir.AluOpType.add

# head pairs (g4, j) whose bias is added on the PE via identity matmuls;
# all other pairs get exp(B) multiplied on the DVE after the exp
PE_BIAS_PAIRS = frozenset({(0, 0)})


DBG = bool(os.environ.get("KDBG"))


def _emit(tc, aps):
    nc = tc.nc
    if DBG:
        x, wqk, wv, wp, bqk, bv, bp, etab, out, dq, dv, dot = aps
    else:
        x, wqk, wv, wp, bqk, bv, bp, etab, out = aps

    with tc.tile_pool(name="persist", bufs=1) as persist:
        # --- constants / weights resident in SBUF ---
        wqk_sb = persist.tile([128, CC, 4, 128], F16)
        nc.sync.dma_start(out=wqk_sb, in_=wqk.rearrange("cc ci jt j -> ci cc jt j"))
        wv_sb = persist.tile([128, CC, 512], F16)
        nc.sync.dma_start(out=wv_sb, in_=wv.rearrange("cc ci v -> ci cc v"))
        wp_sb = persist.tile([128, 4, 256], F16)
        nc.sync.dma_start(out=wp_sb, in_=wp.rearrange("cc ci c -> ci cc c"))
        bqk_sb = persist.tile([128, 4], F32)
        nc.sync.dma_start(out=bqk_sb, in_=bqk.rearrange("jt j -> j jt"))
        bv_sb = persist.tile([128, 512], F32)
        nc.sync.dma_start(out=bv_sb, in_=bv.partition_broadcast(128))
        bp_sb = persist.tile([128, 256], F32)
        nc.sync.dma_start(out=bp_sb, in_=bp.partition_broadcast(128))
        ident = persist.tile([128, 128], F16)
        make_identity(nc, ident)
        negoff = persist.tile([128, 1], F32)
        nc.vector.memset(negoff, -OFF)
        epsv = persist.tile([128, 1], F32)
        nc.vector.memset(epsv, EPS)

        qkT_l = []  # per-b [128, 4, 1024] f16: jt tiles (kT g0, qT g0, kT g1, qT g1)
        v_l = []    # per-b [128, NT, H, 65] f16: V rows + ones column per head
        ot_l = []   # per-b [128, 4, 1024] f16: O.T (dh on partitions, 4 chunks)

        # ---------------- phase 1: LN, xn.T, qkv projections ----------------
        with (
            tc.tile_pool(name="p1", bufs=2) as p1,
            tc.tile_pool(name="p1ps", bufs=2, space="PSUM") as p1ps,
        ):
            # stage 1: all batches' LayerNorms up front — the DVE queue is
            # pure LN with no PE-gated ops, so no head-of-line blocking
            xn_l = []
            for b in range(NB):
                x_sb = p1.tile([128, NT, DIM], F32, tag="x", bufs=4)
                nc.sync.dma_start(
                    out=x_sb, in_=x[b].rearrange("(t p) c -> p t c", p=128)
                )
                xn_sb = p1.tile([128, NT, DIM], F16, tag="xn", bufs=4)
                for t in range(NT):
                    stats = p1.tile([128, 6], F32, tag="stats", bufs=3)
                    nc.vector.bn_stats(out=stats, in_=x_sb[:, t])
                    mv = p1.tile([128, 2], F32, tag="mv", bufs=3)
                    nc.vector.bn_aggr(out=mv, in_=stats)
                    rstd = p1.tile([128, 1], F32, tag="rstd", bufs=3)
                    nc.scalar.activation(
                        out=rstd, in_=mv[:, 1:2],
                        func=mybir.ActivationFunctionType.Sqrt,
                        bias=epsv, scale=1.0,
                    )
                    nc.vector.reciprocal(out=rstd, in_=rstd)
                    nc.vector.tensor_scalar(
                        out=xn_sb[:, t], in0=x_sb[:, t],
                        scalar1=mv[:, 0:1], scalar2=rstd,
                        op0=mybir.AluOpType.subtract, op1=mybir.AluOpType.mult,
                    )
                xn_l.append(xn_sb)

            # stage 2: per-b transposes + projections; transpose drains on
            # GpSimd so ACT stays free and DVE keeps the LN stream
            for b in range(NB):
                xn_sb = xn_l[b]
                xnT = p1.tile([128, CC, N], F16, tag="xnt", bufs=2)
                for cc in range(CC):
                    for t in range(NT):
                        tp = p1ps.tile([128, 128], F16, tag="tp", bufs=2)
                        nc.tensor.transpose(
                            tp, xn_sb[:, t, cc * 128:(cc + 1) * 128], ident
                        )
                        nc.scalar.copy(
                            out=xnT[:, cc, t * 128:(t + 1) * 128], in_=tp
                        )
                # q.T / k.T, packed by 32-row strips per head (zeros padding)
                qkT = persist.tile([128, 4, N], F16, tag="qkT", bufs=NB, name="qkT")
                for jt in range(4):
                    qkp = p1ps.tile([128, N], F32, tag="qkp", bufs=2)
                    for nh in range(2):
                        for cc in range(CC):
                            nc.tensor.matmul(
                                qkp[:, nh * 512:(nh + 1) * 512],
                                lhsT=wqk_sb[:, cc, jt],
                                rhs=xnT[:, cc, nh * 512:(nh + 1) * 512],
                                start=(cc == 0), stop=(cc == CC - 1),
                            )
                    nc.scalar.activation(
                        out=qkT[:, jt], in_=qkp,
                        func=mybir.ActivationFunctionType.Identity,
                        bias=bqk_sb[:, jt:jt + 1], scale=1.0,
                    )
                qkT_l.append(qkT)
                # V (natural layout) + ones column, interleaved per head
                v_sb = persist.tile([128, NT, H, 65], F16, tag="v", bufs=NB,
                                    name="v_sb")
                nc.vector.memset(v_sb[:, :, :, 64:65], 1.0)
                for t in range(NT):
                    vp = p1ps.tile([128, 512], F32, tag="vp", bufs=2)
                    for cc in range(CC):
                        nc.tensor.matmul(
                            vp,
                            lhsT=xnT[:, cc, t * 128:(t + 1) * 128],
                            rhs=wv_sb[:, cc],
                            start=(cc == 0), stop=(cc == CC - 1),
                        )
                    nc.vector.tensor_tensor(
                        out=v_sb[:, t, :, 0:64],
                        in0=vp.rearrange("p (h d) -> p h d", d=64),
                        in1=bv_sb.rearrange("p (h d) -> p h d", d=64),
                        op=ADD,
                    )
                v_l.append(v_sb)
                if DBG:
                    nc.sync.dma_start(out=dq[b], in_=qkT)
                    nc.sync.dma_start(out=dv[b], in_=v_sb)

        # ---------------- phase 2: attention, 4-head groups ----------------
        for b in range(NB):
            ot_l.append(persist.tile([128, 4, N], F16, tag="ot", bufs=NB,
                                     name="ot"))

        with (
            tc.tile_pool(name="p2", bufs=2) as p2,
            tc.tile_pool(name="p2ps", bufs=2, space="PSUM") as p2ps,
        ):
            def emit_proj(b):
                # output projection for batch b; borrows freed PV banks
                for nt in range(NT):
                    y = p2ps.tile([128, 512], F32, tag="o", bufs=4,
                                  name="y_ps")
                    for cc2 in range(4):
                        nc.tensor.matmul(
                            y[:, 0:256],
                            lhsT=ot_l[b][:, cc2, nt * 128:(nt + 1) * 128],
                            rhs=wp_sb[:, cc2],
                            start=(cc2 == 0), stop=(cc2 == 3),
                            skip_group_check=True,
                        )
                    o_sb = p2.tile([128, 256], F32, tag="osb", bufs=4)
                    nc.vector.tensor_tensor(
                        out=o_sb, in0=y[:, 0:256], in1=bp_sb, op=ADD
                    )
                    nc.sync.dma_start(
                        out=out[b].rearrange("(t p) c -> p t c", p=128)[:, nt],
                        in_=o_sb,
                    )

            for g4 in range(2):     # head quad {4g4..4g4+3}
                jtk = 2 * g4        # k.T tile index; q.T at jtk+1
                for nh in range(2):  # n-half (query columns)
                    # bias tiles for this (g4, nh): per (pair j, mc) a
                    # [128, 1024] tile whose halves belong to heads
                    # 4g4+2j and 4g4+2j+1 — matching the ps tile layout.
                    # g4==0: additive B (identity-matmul into score PSUM);
                    # g4==1: exp(B) (DVE multiply after the exp).
                    e_tiles = {}
                    for j in range(2):
                        for mc in range(MC):
                            et = p2.tile([128, N], F16, tag="e", bufs=20,
                                         name="et")
                            nc.sync.dma_start(
                                out=et, in_=etab[g4, j, mc, :, nh]
                            )
                            e_tiles[(j, mc)] = et

                    for b in range(NB):
                        # PV accumulators: per head [65, 512] in a
                        # [128, 512] one-bank tile; row 64 = softmax sums
                        o_ts = [
                            p2ps.tile([128, 512], F32, tag="o", bufs=4,
                                      name="o_ts")
                            for _ in range(4)
                        ]
                        s_tiles = {}

                        def emit_scores(mc, b=b, s_tiles=s_tiles):
                            # 4 score matmuls back-to-back, one per head in
                            # its own 32-row strip -> they stream
                            # concurrently in the PE array; additive-bias
                            # identity matmuls (head quad 0) follow
                            s_ab = [
                                p2ps.tile([128, N], F32, tag="s", bufs=2,
                                          name="s_ps")
                                for _ in range(2)
                            ]
                            for i in range(4):
                                strip = 32 * i
                                s = s_ab[i // 2]
                                col = (i % 2) * 512
                                nc.tensor.matmul(
                                    s[:, col:col + 512],
                                    lhsT=qkT_l[b][strip:strip + KD, jtk,
                                                  mc * 128:(mc + 1) * 128],
                                    rhs=qkT_l[b][strip:strip + KD, jtk + 1,
                                                 nh * 512:(nh + 1) * 512],
                                    start=True,
                                    stop=(g4, i // 2) not in PE_BIAS_PAIRS,
                                    skip_group_check=True,
                                    tile_position=(strip, 0),
                                )
                            for i in range(4):
                                if (g4, i // 2) not in PE_BIAS_PAIRS:
                                    continue
                                s = s_ab[i // 2]
                                col = (i % 2) * 512
                                nc.tensor.matmul(
                                    s[:, col:col + 512],
                                    lhsT=ident,
                                    rhs=e_tiles[(i // 2, mc)][
                                        :, col:col + 512],
                                    start=False, stop=True,
                                    skip_group_check=True,
                                )
                            s_tiles[mc] = s_ab

                        emit_scores(0)
                        for mc in range(MC):
                            # keep the PE queue fed: next mc's scores go
                            # ahead of this mc's exp-gated consumers
                            if mc + 1 < MC:
                                emit_scores(mc + 1)
                            s_ab = s_tiles.pop(mc)
                            ps_ab = []
                            for j in range(2):
                                ps = p2.tile([128, N], F16, tag="ps", bufs=4,
                                             name="ps")
                                nc.scalar.activation(
                                    out=ps, in_=s_ab[j],
                                    func=mybir.ActivationFunctionType.Exp,
                                    bias=negoff, scale=1.0,
                                )
                                if (g4, j) not in PE_BIAS_PAIRS:
                                    # multiplicative bias exp(B), spread
                                    # over DVE and the idle GpSimd
                                    pm = p2.tile([128, N], F16, tag="pm",
                                                 bufs=4, name="pm")
                                    eng = (nc.gpsimd if (g4, j) == (1, 1)
                                           else nc.vector)
                                    eng.tensor_tensor(
                                        out=pm, in0=ps,
                                        in1=e_tiles[(j, mc)],
                                        op=MULT,
                                    )
                                    ps = pm
                                ps_ab.append(ps)
                            for i in range(4):
                                col = (i % 2) * 512
                                nc.tensor.matmul(
                                    o_ts[i][0:65, :],
                                    lhsT=v_l[b][:, mc, 4 * g4 + i],
                                    rhs=ps_ab[i // 2][:, col:col + 512],
                                    start=(mc == 0), stop=(mc == MC - 1),
                                    skip_group_check=True,
                                )
                        # normalize: recip of sums row -> broadcast -> mult
                        # sums rows must reach SBUF before the custom-DVE
                        # reciprocal (PSUM-source custom ops misbehave on
                        # HW); batch the 4 heads into one collector whose
                        # rows sit at 32-aligned partition bases, one
                        # reciprocal covers all 4
                        c97 = p2.tile([97, 512], F32, tag="sm", bufs=2)
                        for i in range(4):
                            nc.vector.tensor_copy(
                                out=c97[32 * i:32 * i + 1], in_=o_ts[i][64:65]
                            )
                        r97 = p2.tile([97, 512], F32, tag="r32", bufs=2)
                        nc.vector.reciprocal_approx_fast(out=r97, in_=c97)
                        for i in range(4):
                            h = 4 * g4 + i
                            # broadcast source must be f16 at partition 0
                            r1 = p2.tile([1, 512], F16, tag="r1", bufs=4)
                            nc.vector.tensor_copy(
                                out=r1, in_=r97[32 * i:32 * i + 1]
                            )
                            rb = p2.tile([64, 512], F16, tag="rb", bufs=3)
                            nc.gpsimd.partition_broadcast(rb, r1)
                            # drain + normalize fused: DVE reads the PV
                            # accumulator straight from PSUM
                            nc.vector.tensor_tensor(
                                out=ot_l[b][64 * (h % 2):64 * (h % 2) + 64,
                                            h // 2,
                                            nh * 512:(nh + 1) * 512],
                                in0=o_ts[i][0:64], in1=rb, op=MULT,
                            )
                        if g4 == 1 and nh == 1:
                            if DBG:
                                nc.sync.dma_start(out=dot[b], in_=ot_l[b])
                            emit_proj(b)


def build_module():
    nc = bacc.Bacc(
        "TRN2",
        target_bir_lowering=False,
        debug=False,
        enable_asserts=False,
        num_devices=NCORES,
    )
    x_t = nc.dram_tensor("x", [NB, N, DIM], F32, kind="ExternalInput")
    wqk_t = nc.dram_tensor("wqk", [CC, 128, 4, 128], F16, kind="ExternalInput")
    wv_t = nc.dram_tensor("wv", [CC, 128, 512], F16, kind="ExternalInput")
    wp_t = nc.dram_tensor("wp", [4, 128, 256], F16, kind="ExternalInput")
    bqk_t = nc.dram_tensor("bqk", [4, 128], F32, kind="ExternalInput")
    bv_t = nc.dram_tensor("bv", [512], F32, kind="ExternalInput")
    bp_t = nc.dram_tensor("bp", [256], F32, kind="ExternalInput")
    e_t = nc.dram_tensor("etab", [2, 2, MC, 128, 2, N], F16,
                         kind="ExternalInput")
    out_t = nc.dram_tensor("out", [NB, N, DIM], F32, kind="ExternalOutput")

    ts = [x_t, wqk_t, wv_t, wp_t, bqk_t, bv_t, bp_t, e_t, out_t]
    if DBG:
        ts.append(nc.dram_tensor("dq", [NB, 128, 4, N], F16, kind="ExternalOutput"))
        ts.append(nc.dram_tensor("dv", [NB, 128, NT, H, 65], F16, kind="ExternalOutput"))
        ts.append(nc.dram_tensor("dot", [NB, 128, 4, N], F16, kind="ExternalOutput"))
    aps = [t.ap() for t in ts]
    with tile.TileContext(nc) as tc:
        _emit(tc, aps)
    nc.compile()
    return nc


def prep_inputs(inputs):
    """Host-side prep: fold norm affine + scale into weights, pack q/k rows
    into 32-row strips for PE row-tiling, and build the bias tables
    (additive B for heads 0-3, exp(B) for heads 4-7)."""
    x = np.asarray(inputs["x"], np.float32)
    norm_w = np.asarray(inputs["norm_w"], np.float32)
    norm_b = np.asarray(inputs["norm_b"], np.float32)
    qkv_w = np.asarray(inputs["qkv_w"], np.float32)
    qkv_b = np.asarray(inputs["qkv_b"], np.float32)
    proj_w = np.asarray(inputs["proj_w"], np.float32)
    proj_b = np.asarray(inputs["proj_b"], np.float32)
    ab = np.asarray(inputs["attn_biases"], np.float32)
    bi = np.asarray(inputs["bias_idxs"], np.int64)

    scale = KD ** -0.5
    wr = qkv_w.reshape(H, 2 * KD + D, DIM)
    br = qkv_b.reshape(H, 2 * KD + D)
    # fold norm_w into weights, norm_b into biases
    w_eff = wr * norm_w[None, None, :]
    b_eff = br + wr @ norm_b
    w_q = w_eff[:, :KD] * scale
    b_q = b_eff[:, :KD] * scale
    w_k = w_eff[:, KD:2 * KD]
    b_k = b_eff[:, KD:2 * KD]
    w_v = w_eff[:, 2 * KD:]
    b_v = b_eff[:, 2 * KD:]

    wqk = np.zeros((CC, 128, 4, 128), np.float16)
    bqk = np.zeros((4, 128), np.float32)
    for jt in range(4):
        kind_q = jt % 2 == 1
        hg = jt // 2
        w_src = w_q if kind_q else w_k
        b_src = b_q if kind_q else b_k
        for hp in range(4):
            h = hg * 4 + hp
            w_jc = w_src[h]  # [KD, DIM]
            for cc in range(CC):
                wqk[cc, :, jt, 32 * hp:32 * hp + KD] = (
                    w_jc[:, cc * 128:(cc + 1) * 128].T.astype(np.float16)
                )
            bqk[jt, 32 * hp:32 * hp + KD] = b_src[h]

    wv = np.zeros((CC, 128, 512), np.float16)
    for cc in range(CC):
        # [512(h,d), 128] -> [128, 512]
        wv[cc] = w_v.reshape(512, DIM)[:, cc * 128:(cc + 1) * 128].T.astype(np.float16)
    bv = b_v.reshape(512).astype(np.float32)

    wp = np.zeros((4, 128, 256), np.float16)
    for cc2 in range(4):
        wp[cc2] = proj_w[:, cc2 * 128:(cc2 + 1) * 128].T.astype(np.float16)
    bp = proj_b.astype(np.float32)

    # relative-position bias tables, [g4, pair j, mc, p(m), nh, 1024(n)]
    # with the two heads of a pair in the two 512-column halves; additive B
    # for head quad 0, multiplicative exp(B) for head quad 1 (B symmetric)
    B = ab[:, bi]  # [H, n, m] == [H, m, n]
    etab = np.zeros((2, 2, MC, 128, 2, N), np.float16)
    for g4 in range(2):
        for j in range(2):
            for k in range(2):
                h = 4 * g4 + 2 * j + k
                M = B[h] if (g4, j) in PE_BIAS_PAIRS else np.exp(B[h])
                for mc in range(MC):
                    for nh in range(2):
                        etab[g4, j, mc, :, nh, k * 512:(k + 1) * 512] = (
                            M[mc * 128:(mc + 1) * 128,
                              nh * 512:(nh + 1) * 512].astype(np.float16)
                        )

    shared = {
        "wqk": wqk, "wv": wv, "wp": wp,
        "bqk": bqk, "bv": bv, "bp": bp, "etab": etab,
    }
    in_maps = []
    for c in range(NCORES):
        m = dict(shared)
        m["x"] = np.ascontiguousarray(x[c * NB:(c + 1) * NB])
        in_maps.append(m)
    return in_maps


_NC_CACHE = None


def _get_nc():
    global _NC_CACHE
    if _NC_CACHE is None:
        _NC_CACHE = build_module()
    return _NC_CACHE


def run(inputs, **spmd_kwargs):
    nc = _get_nc()
    in_maps = prep_inputs(inputs)
    res = bass_utils.run_bass_kernel_spmd(
        nc, in_maps, core_ids=list(range(NCORES)), **spmd_kwargs
    )
    out = np.concatenate([res.results[c]["out"] for c in range(NCORES)], axis=0)
    return out.astype(np.float32), res


def kernel(**inputs):
    out, _ = run(inputs)
    return out


if __name__ == "__main__":
    print("building module...")
    nc = _get_nc()
    print("module built ok")


# revision 20
# speedup vs baseline: 1.1489x; 1.1489x over previous
"""Fused sparse-attention kernel for TRN2, SPMD over 8 NeuronCores.

Sharding: data-parallel over batch (32 -> 4 per core). Per core, the full
block (LayerNorm -> fused qkv -> per-head attention with gathered relative
position bias -> proj) is computed on-chip; attention probabilities never
touch HBM.

v2 engine-balance restructure (baseline was PE-bound at 89% busy with ACT
at 66%):

* Scores are emitted four heads at a time into four distinct 32-row PE
  strips (tile_position row groups), so the K=16 QK^T matmuls stream
  concurrently instead of serially (the baseline interleaved full-row
  identity matmuls between them, which serialized everything).
* The gathered relative-position bias is split by head group: heads 0-3
  keep the additive-B identity-matmul accumulation into the score PSUM
  (PE), heads 4-7 multiply exp(B) into the exp'd probabilities on the DVE
  (f16 tensor_tensor at 2x rate). This splits ~110us of bias work across
  two engines instead of loading either fully.
* ACT runs only exp (fused [128,1024] two-head tiles) plus the tiny LN
  sqrt; PSUM drains move to GpSimd, the qkv-bias drain to the DVE.
* Softmax normalization: ones-column in V gives row sums in PSUM row 64;
  a per-head reciprocal_approx_fast writes a [1,512] f16 row at partition
  base 0, GpSimd broadcasts it, DVE multiplies (f16 2x).
* The output projection for batch b is emitted right after its attention
  completes (inside the last head-group pass) and borrows the freed PV
  PSUM banks, so proj matmuls fill PE gaps while ACT streams the next
  batch's exps.
"""

import os
import sys

import numpy as np

for _p in ("/opt/trn_rl_repo", "/root/.axon_site/_ro/trn_rl_repo"):
    if os.path.isdir(_p) and _p not in sys.path:
        sys.path.insert(0, _p)

import concourse.bacc as bacc
import concourse.tile as tile
from concourse import bass_utils, mybir
from concourse.masks import make_identity

F32 = mybir.dt.float32
F16 = mybir.dt.float16

NCORES = 8
B_TOTAL = 32
NB = B_TOTAL // NCORES  # local batch per core
N = 1024
NT = 8        # 128-row tiles over n
DIM = 256
CC = 2        # 128-row chunks over DIM
H = 8
KD = 16
D = 64
MC = 8        # 128-row chunks over m
EPS = 1e-5
OFF = float(4.0 * np.log(2.0))  # exp offset for fp16 headroom (cancels)

MULT = mybir.AluOpType.mult
ADD = mybir.AluOpType.add

# head pairs (g4, j) whose bias is added on the PE via identity matmuls;
# all other pairs get exp(B) multiplied on the DVE after the exp
PE_BIAS_PAIRS = frozenset({(0, 0), (1, 1)})


DBG = bool(os.environ.get("KDBG"))


def _emit(tc, aps):
    nc = tc.nc
    if DBG:
        x, wqk, wv, wp, bqk, bv, bp, etab, out, dq, dv, dot = aps
    else:
        x, wqk, wv, wp, bqk, bv, bp, etab, out = aps

    with tc.tile_pool(name="persist", bufs=1) as persist:
        # --- constants / weights resident in SBUF ---
        wqk_sb = persist.tile([128, CC, 4, 128], F16)
        nc.sync.dma_start(out=wqk_sb, in_=wqk.rearrange("cc ci jt j -> ci cc jt j"))
        wv_sb = persist.tile([128, CC, 512], F16)
        nc.sync.dma_start(out=wv_sb, in_=wv.rearrange("cc ci v -> ci cc v"))
        wp_sb = persist.tile([128, 4, 256], F16)
        nc.sync.dma_start(out=wp_sb, in_=wp.rearrange("cc ci c -> ci cc c"))
        bqk_sb = persist.tile([128, 4], F32)
        nc.sync.dma_start(out=bqk_sb, in_=bqk.rearrange("jt j -> j jt"))
        bv_sb = persist.tile([128, 512], F32)
        nc.sync.dma_start(out=bv_sb, in_=bv.partition_broadcast(128))
        bp_sb = persist.tile([128, 256], F32)
        nc.sync.dma_start(out=bp_sb, in_=bp.partition_broadcast(128))
        ident = persist.tile([128, 128], F16)
        make_identity(nc, ident)
        negoff = persist.tile([128, 1], F32)
        nc.vector.memset(negoff, -OFF)
        epsv = persist.tile([128, 1], F32)
        nc.vector.memset(epsv, EPS)

        qkT_l = []  # per-b [128, 4, 1024] f16: jt tiles (kT g0, qT g0, kT g1, qT g1)
        v_l = []    # per-b [128, NT, H, 65] f16: V rows + ones column per head
        ot_l = []   # per-b [128, 4, 1024] f16: O.T (dh on partitions, 4 chunks)

        # ---------------- phase 1: LN, xn.T, qkv projections ----------------
        with (
            tc.tile_pool(name="p1", bufs=2) as p1,
            tc.tile_pool(name="p1ps", bufs=2, space="PSUM") as p1ps,
        ):
            # stage 1: all batches' LayerNorms up front — the DVE queue is
            # pure LN with no PE-gated ops, so no head-of-line blocking
            xn_l = []
            for b in range(NB):
                x_sb = p1.tile([128, NT, DIM], F32, tag="x", bufs=4)
                nc.sync.dma_start(
                    out=x_sb, in_=x[b].rearrange("(t p) c -> p t c", p=128)
                )
                xn_sb = p1.tile([128, NT, DIM], F16, tag="xn", bufs=4)
                for t in range(NT):
                    stats = p1.tile([128, 6], F32, tag="stats", bufs=3)
                    nc.vector.bn_stats(out=stats, in_=x_sb[:, t])
                    mv = p1.tile([128, 2], F32, tag="mv", bufs=3)
                    nc.vector.bn_aggr(out=mv, in_=stats)
                    rstd = p1.tile([128, 1], F32, tag="rstd", bufs=3)
                    nc.scalar.activation(
                        out=rstd, in_=mv[:, 1:2],
                        func=mybir.ActivationFunctionType.Sqrt,
                        bias=epsv, scale=1.0,
                    )
                    nc.vector.reciprocal(out=rstd, in_=rstd)
                    nc.vector.tensor_scalar(
                        out=xn_sb[:, t], in0=x_sb[:, t],
                        scalar1=mv[:, 0:1], scalar2=rstd,
                        op0=mybir.AluOpType.subtract, op1=mybir.AluOpType.mult,
                    )
                xn_l.append(xn_sb)

            # stage 2: per-b transposes + projections; transpose drains on
            # GpSimd so ACT stays free and DVE keeps the LN stream
            for b in range(NB):
                xn_sb = xn_l[b]
                xnT = p1.tile([128, CC, N], F16, tag="xnt", bufs=2)
                for cc in range(CC):
                    for t in range(NT):
                        # SBUF->SBUF transpose on the DMA xbar: no PE time,
                        # no PSUM bank, no drain op
                        nc.sync.dma_start_transpose(
                            out=xnT[:, cc, t * 128:(t + 1) * 128],
                            in_=xn_sb[:, t, cc * 128:(cc + 1) * 128],
                        )
                # q.T / k.T, packed by 32-row strips per head (zeros padding)
                qkT = persist.tile([128, 4, N], F16, tag="qkT", bufs=NB, name="qkT")
                for jt in range(4):
                    qkp = p1ps.tile([128, N], F32, tag="qkp", bufs=2)
                    for nh in range(2):
                        for cc in range(CC):
                            nc.tensor.matmul(
                                qkp[:, nh * 512:(nh + 1) * 512],
                                lhsT=wqk_sb[:, cc, jt],
                                rhs=xnT[:, cc, nh * 512:(nh + 1) * 512],
                                start=(cc == 0), stop=(cc == CC - 1),
                            )
                    nc.scalar.activation(
                        out=qkT[:, jt], in_=qkp,
                        func=mybir.ActivationFunctionType.Identity,
                        bias=bqk_sb[:, jt:jt + 1], scale=1.0,
                    )
                qkT_l.append(qkT)
                # V (natural layout) + ones column, interleaved per head
                v_sb = persist.tile([128, NT, H, 65], F16, tag="v", bufs=NB,
                                    name="v_sb")
                nc.vector.memset(v_sb[:, :, :, 64:65], 1.0)
                for t in range(NT):
                    vp = p1ps.tile([128, 512], F32, tag="vp", bufs=2)
                    for cc in range(CC):
                        nc.tensor.matmul(
                            vp,
                            lhsT=xnT[:, cc, t * 128:(t + 1) * 128],
                            rhs=wv_sb[:, cc],
                            start=(cc == 0), stop=(cc == CC - 1),
                        )
                    nc.vector.tensor_tensor(
                        out=v_sb[:, t, :, 0:64],
                        in0=vp.rearrange("p (h d) -> p h d", d=64),
                        in1=bv_sb.rearrange("p (h d) -> p h d", d=64),
                        op=ADD,
                    )
                v_l.append(v_sb)
                if DBG:
                    nc.sync.dma_start(out=dq[b], in_=qkT)
                    nc.sync.dma_start(out=dv[b], in_=v_sb)

        # ---------------- phase 2: attention, 4-head groups ----------------
        for b in range(NB):
            ot_l.append(persist.tile([128, 4, N], F16, tag="ot", bufs=NB,
                                     name="ot"))

        with (
            tc.tile_pool(name="p2", bufs=2) as p2,
            tc.tile_pool(name="p2ps", bufs=2, space="PSUM") as p2ps,
        ):
            def emit_proj(b):
                # output projection for batch b; borrows freed PV banks
                for nt in range(NT):
                    y = p2ps.tile([128, 512], F32, tag="o", bufs=4,
                                  name="y_ps")
                    for cc2 in range(4):
                        nc.tensor.matmul(
                            y[:, 0:256],
                            lhsT=ot_l[b][:, cc2, nt * 128:(nt + 1) * 128],
                            rhs=wp_sb[:, cc2],
                            start=(cc2 == 0), stop=(cc2 == 3),
                            skip_group_check=True,
                        )
                    o_sb = p2.tile([128, 256], F32, tag="osb", bufs=4)
                    nc.vector.tensor_tensor(
                        out=o_sb, in0=y[:, 0:256], in1=bp_sb, op=ADD
                    )
                    nc.sync.dma_start(
                        out=out[b].rearrange("(t p) c -> p t c", p=128)[:, nt],
                        in_=o_sb,
                    )

            for g4 in range(2):     # head quad {4g4..4g4+3}
                jtk = 2 * g4        # k.T tile index; q.T at jtk+1
                for nh in range(2):  # n-half (query columns)
                    # bias tiles for this (g4, nh): per (pair j, mc) a
                    # [128, 1024] tile whose halves belong to heads
                    # 4g4+2j and 4g4+2j+1 — matching the ps tile layout.
                    # g4==0: additive B (identity-matmul into score PSUM);
                    # g4==1: exp(B) (DVE multiply after the exp).
                    e_tiles = {}
                    for j in range(2):
                        for mc in range(MC):
                            et = p2.tile([128, N], F16, tag="e", bufs=20,
                                         name="et")
                            nc.sync.dma_start(
                                out=et, in_=etab[g4, j, mc, :, nh]
                            )
                            e_tiles[(j, mc)] = et

                    for b in range(NB):
                        # PV accumulators: per head [65, 512] in a
                        # [128, 512] one-bank tile; row 64 = softmax sums
                        o_ts = [
                            p2ps.tile([128, 512], F32, tag="o", bufs=4,
                                      name="o_ts")
                            for _ in range(4)
                        ]
                        s_tiles = {}

                        def emit_scores(mc, b=b, s_tiles=s_tiles):
                            # 4 score matmuls back-to-back, one per head in
                            # its own 32-row strip -> they stream
                            # concurrently in the PE array; additive-bias
                            # identity matmuls (head quad 0) follow
                            s_ab = [
                                p2ps.tile([128, N], F32, tag="s", bufs=2,
                                          name="s_ps")
                                for _ in range(2)
                            ]
                            for i in range(4):
                                strip = 32 * i
                                s = s_ab[i // 2]
                                col = (i % 2) * 512
                                nc.tensor.matmul(
                                    s[:, col:col + 512],
                                    lhsT=qkT_l[b][strip:strip + KD, jtk,
                                                  mc * 128:(mc + 1) * 128],
                                    rhs=qkT_l[b][strip:strip + KD, jtk + 1,
                                                 nh * 512:(nh + 1) * 512],
                                    start=True,
                                    stop=(g4, i // 2) not in PE_BIAS_PAIRS,
                                    skip_group_check=True,
                                    tile_position=(strip, 0),
                                )
                            for i in range(4):
                                if (g4, i // 2) not in PE_BIAS_PAIRS:
                                    continue
                                s = s_ab[i // 2]
                                col = (i % 2) * 512
                                nc.tensor.matmul(
                                    s[:, col:col + 512],
                                    lhsT=ident,
                                    rhs=e_tiles[(i // 2, mc)][
                                        :, col:col + 512],
                                    start=False, stop=True,
                                    skip_group_check=True,
                                )
                            s_tiles[mc] = s_ab

                        emit_scores(0)
                        for mc in range(MC):
                            # keep the PE queue fed: next mc's scores go
                            # ahead of this mc's exp-gated consumers
                            if mc + 1 < MC:
                                emit_scores(mc + 1)
                            s_ab = s_tiles.pop(mc)
                            ps_ab = []
                            for j in range(2):
                                ps = p2.tile([128, N], F16, tag="ps", bufs=4,
                                             name="ps")
                                nc.scalar.activation(
                                    out=ps, in_=s_ab[j],
                                    func=mybir.ActivationFunctionType.Exp,
                                    bias=negoff, scale=1.0,
                                )
                                if (g4, j) not in PE_BIAS_PAIRS:
                                    # multiplicative bias exp(B), spread
                                    # over DVE and the idle GpSimd
                                    pm = p2.tile([128, N], F16, tag="pm",
                                                 bufs=4, name="pm")
                                    nc.vector.tensor_tensor(
                                        out=pm, in0=ps,
                                        in1=e_tiles[(j, mc)],
                                        op=MULT,
                                    )
                                    ps = pm
                                ps_ab.append(ps)
                            for i in range(4):
                                col = (i % 2) * 512
                                nc.tensor.matmul(
                                    o_ts[i][0:65, :],
                                    lhsT=v_l[b][:, mc, 4 * g4 + i],
                                    rhs=ps_ab[i // 2][:, col:col + 512],
                                    start=(mc == 0), stop=(mc == MC - 1),
                                    skip_group_check=True,
                                )
                        # normalize: recip of sums row -> broadcast -> mult
                        # sums rows must reach SBUF before the custom-DVE
                        # reciprocal (PSUM-source custom ops misbehave on
                        # HW); batch the 4 heads into one collector whose
                        # rows sit at 32-aligned partition bases, one
                        # reciprocal covers all 4
                        c97 = p2.tile([97, 512], F32, tag="sm", bufs=2)
                        for i in range(4):
                            nc.vector.tensor_copy(
                                out=c97[32 * i:32 * i + 1], in_=o_ts[i][64:65]
                            )
                        r97 = p2.tile([97, 512], F32, tag="r32", bufs=2)
                        nc.vector.reciprocal_approx_fast(out=r97, in_=c97)
                        for i in range(4):
                            h = 4 * g4 + i
                            # broadcast source must be f16 at partition 0
                            r1 = p2.tile([1, 512], F16, tag="r1", bufs=4)
                            nc.vector.tensor_copy(
                                out=r1, in_=r97[32 * i:32 * i + 1]
                            )
                            rb = p2.tile([64, 512], F16, tag="rb", bufs=3)
                            nc.gpsimd.partition_broadcast(rb, r1)
                            # drain + normalize fused: DVE reads the PV
                            # accumulator straight from PSUM
                            nc.vector.tensor_tensor(
                                out=ot_l[b][64 * (h % 2):64 * (h % 2) + 64,
                                            h // 2,
                                            nh * 512:(nh + 1) * 512],
                                in0=o_ts[i][0:64], in1=rb, op=MULT,
                            )
                        if g4 == 1 and nh == 1:
                            if DBG:
                                nc.sync.dma_start(out=dot[b], in_=ot_l[b])
                            emit_proj(b)


def build_module():
    nc = bacc.Bacc(
        "TRN2",
        target_bir_lowering=False,
        debug=False,
        enable_asserts=False,
        num_devices=NCORES,
    )
    x_t = nc.dram_tensor("x", [NB, N, DIM], F32, kind="ExternalInput")
    wqk_t = nc.dram_tensor("wqk", [CC, 128, 4, 128], F16, kind="ExternalInput")
    wv_t = nc.dram_tensor("wv", [CC, 128, 512], F16, kind="ExternalInput")
    wp_t = nc.dram_tensor("wp", [4, 128, 256], F16, kind="ExternalInput")
    bqk_t = nc.dram_tensor("bqk", [4, 128], F32, kind="ExternalInput")
    bv_t = nc.dram_tensor("bv", [512], F32, kind="ExternalInput")
    bp_t = nc.dram_tensor("bp", [256], F32, kind="ExternalInput")
    e_t = nc.dram_tensor("etab", [2, 2, MC, 128, 2, N], F16,
                         kind="ExternalInput")
    out_t = nc.dram_tensor("out", [NB, N, DIM], F32, kind="ExternalOutput")

    ts = [x_t, wqk_t, wv_t, wp_t, bqk_t, bv_t, bp_t, e_t, out_t]
    if DBG:
        ts.append(nc.dram_tensor("dq", [NB, 128, 4, N], F16, kind="ExternalOutput"))
        ts.append(nc.dram_tensor("dv", [NB, 128, NT, H, 65], F16, kind="ExternalOutput"))
        ts.append(nc.dram_tensor("dot", [NB, 128, 4, N], F16, kind="ExternalOutput"))
    aps = [t.ap() for t in ts]
    with tile.TileContext(nc) as tc:
        _emit(tc, aps)
    nc.compile()
    return nc


def prep_inputs(inputs):
    """Host-side prep: fold norm affine + scale into weights, pack q/k rows
    into 32-row strips for PE row-tiling, and build the bias tables
    (additive B for heads 0-3, exp(B) for heads 4-7)."""
    x = np.asarray(inputs["x"], np.float32)
    norm_w = np.asarray(inputs["norm_w"], np.float32)
    norm_b = np.asarray(inputs["norm_b"], np.float32)
    qkv_w = np.asarray(inputs["qkv_w"], np.float32)
    qkv_b = np.asarray(inputs["qkv_b"], np.float32)
    proj_w = np.asarray(inputs["proj_w"], np.float32)
    proj_b = np.asarray(inputs["proj_b"], np.float32)
    ab = np.asarray(inputs["attn_biases"], np.float32)
    bi = np.asarray(inputs["bias_idxs"], np.int64)

    scale = KD ** -0.5
    wr = qkv_w.reshape(H, 2 * KD + D, DIM)
    br = qkv_b.reshape(H, 2 * KD + D)
    # fold norm_w into weights, norm_b into biases
    w_eff = wr * norm_w[None, None, :]
    b_eff = br + wr @ norm_b
    w_q = w_eff[:, :KD] * scale
    b_q = b_eff[:, :KD] * scale
    w_k = w_eff[:, KD:2 * KD]
    b_k = b_eff[:, KD:2 * KD]
    w_v = w_eff[:, 2 * KD:]
    b_v = b_eff[:, 2 * KD:]

    wqk = np.zeros((CC, 128, 4, 128), np.float16)
    bqk = np.zeros((4, 128), np.float32)
    for jt in range(4):
        kind_q = jt % 2 == 1
        hg = jt // 2
        w_src = w_q if kind_q else w_k
        b_src = b_q if kind_q else b_k
        for hp in range(4):
            h = hg * 4 + hp
            w_jc = w_src[h]  # [KD, DIM]
            for cc in range(CC):
                wqk[cc, :, jt, 32 * hp:32 * hp + KD] = (
                    w_jc[:, cc * 128:(cc + 1) * 128].T.astype(np.float16)
                )
            bqk[jt, 32 * hp:32 * hp + KD] = b_src[h]

    wv = np.zeros((CC, 128, 512), np.float16)
    for cc in range(CC):
        # [512(h,d), 128] -> [128, 512]
        wv[cc] = w_v.reshape(512, DIM)[:, cc * 128:(cc + 1) * 128].T.astype(np.float16)
    bv = b_v.reshape(512).astype(np.float32)

    wp = np.zeros((4, 128, 256), np.float16)
    for cc2 in range(4):
        wp[cc2] = proj_w[:, cc2 * 128:(cc2 + 1) * 128].T.astype(np.float16)
    bp = proj_b.astype(np.float32)

    # relative-position bias tables, [g4, pair j, mc, p(m), nh, 1024(n)]
    # with the two heads of a pair in the two 512-column halves; additive B
    # for head quad 0, multiplicative exp(B) for head quad 1 (B symmetric)
    B = ab[:, bi]  # [H, n, m] == [H, m, n]
    etab = np.zeros((2, 2, MC, 128, 2, N), np.float16)
    for g4 in range(2):
        for j in range(2):
            for k in range(2):
                h = 4 * g4 + 2 * j + k
                M = B[h] if (g4, j) in PE_BIAS_PAIRS else np.exp(B[h])
                for mc in range(MC):
                    for nh in range(2):
                        etab[g4, j, mc, :, nh, k * 512:(k + 1) * 512] = (
                            M[mc * 128:(mc + 1) * 128,
                              nh * 512:(nh + 1) * 512].astype(np.float16)
                        )

    shared = {
        "wqk": wqk, "wv": wv, "wp": wp,
        "bqk": bqk, "bv": bv, "bp": bp, "etab": etab,
    }
    in_maps = []
    for c in range(NCORES):
        m = dict(shared)
        m["x"] = np.ascontiguousarray(x[c * NB:(c + 1) * NB])
        in_maps.append(m)
    return in_maps


_NC_CACHE = None


def _get_nc():
    global _NC_CACHE
    if _NC_CACHE is None:
        _NC_CACHE = build_module()
    return _NC_CACHE


def run(inputs, **spmd_kwargs):
    nc = _get_nc()
    in_maps = prep_inputs(inputs)
    res = bass_utils.run_bass_kernel_spmd(
        nc, in_maps, core_ids=list(range(NCORES)), **spmd_kwargs
    )
    out = np.concatenate([res.results[c]["out"] for c in range(NCORES)], axis=0)
    return out.astype(np.float32), res


def kernel(**inputs):
    out, _ = run(inputs)
    return out


if __name__ == "__main__":
    print("building module...")
    nc = _get_nc()
    print("module built ok")
